# Initial kernel scaffold
#
import sys, time
sys.path.insert(0, "/opt/trn_rl_repo")
import numpy as np
from concourse import bass, bacc, mybir, tile
from concourse.bass_utils import run_bass_kernel_spmd

# Problem constants (nn_Memory_88656714925588)
B, CK, CV = 1, 64, 256
H, W, T = 64, 64, 8
NE = H * W * T            # 32768
Q = H * W                 # 4096
NC = 8                    # cores
NE_LOC = NE // NC         # 4096 memory elements per core
Q_LOC = Q // NC           # 512 queries per core in phase 3
TOPK = 20
NGRP = 3                  # groups per query-tile in phase 1
GB = [0, 1366, 2732, 4096]  # uneven group bounds over NE_LOC
NCAND = NGRP * 8          # 24 candidates per (query, core)
NSLOT = NC * NCAND        # 256 candidates per query after all-gather
NQT = Q // 128            # 32 query tiles in phase 1
NQT3 = Q_LOC // 128       # 4 query tiles per core in phase 3
F32 = mybir.dt.float32
U32 = mybir.dt.uint32
NEG = -1e30

_prog_cache = {}


def _build_program(phases="123"):
    if phases in _prog_cache:
        return _prog_cache[phases]
    nc = bacc.Bacc()
    qTa = nc.dram_tensor("qTa", [CK + 1, Q], F32, kind="ExternalInput")
    mkA = nc.dram_tensor("mkA", [CK + 1, NE_LOC], F32, kind="ExternalInput")
    vT = nc.dram_tensor("vT", [NE, 2 * CV], F32, kind="ExternalInput")
    gnc = nc.dram_tensor("gnc", [128, NCAND], F32, kind="ExternalInput")
    prow256 = nc.dram_tensor("prow256", [128, 1], F32, kind="ExternalInput")
    out = nc.dram_tensor("out", [Q_LOC, 2 * CV], F32, kind="ExternalOutput")

    with tile.TileContext(nc) as tc:
        with tc.tile_pool(name="sbuf", bufs=2) as pool, \
             tc.tile_pool(name="deep", bufs=10) as deep, \
             tc.tile_pool(name="affp", bufs=4) as affp, \
             tc.tile_pool(name="cst", bufs=1) as cst, \
             tc.tile_pool(name="psum", bufs=2, space="PSUM") as psum, \
             tc.tile_pool(name="dram", bufs=2, space="DRAM") as dram:

            qt = cst.tile([CK + 1, Q], F32)
            mkt = cst.tile([CK + 1, NE_LOC], F32)
            # chunked loads: first matmul needs only mkt[:, :512] and
            # qt[:, :128], so let compute start before the full MB lands
            for ci in range(8):
                nc.sync.dma_start(
                    out=mkt[:, ci * 512:(ci + 1) * 512],
                    in_=mkA[:, ci * 512:(ci + 1) * 512])
            for ci in range(4):
                nc.sync.dma_start(
                    out=qt[:, ci * 1024:(ci + 1) * 1024],
                    in_=qTa[:, ci * 1024:(ci + 1) * 1024])
            gb = cst.tile([128, NCAND], F32)
            nc.sync.dma_start(out=gb[:], in_=gnc[:])
            pr256 = cst.tile([128, 1], F32)
            nc.sync.dma_start(out=pr256[:], in_=prow256[:])

            candL = dram.tile([Q, 2 * NCAND], F32)
            candX = dram.tile([Q, 2 * NCAND], F32)

            # ---------------- Phase 3: merge + readout (q-sharded) --------
            def phase3(tt):
                cG = pool.tile([128, NC * 2 * NCAND], F32, tag="cG")
                nc.sync.dma_start(
                    out=cG[:],
                    in_=candX[tt * NC * 128:(tt + 1) * NC * 128, :]
                    .rearrange("(g p) c -> p g c", p=128))
                candQ = dram.tile([128 * NSLOT, 2], F32, tag="candQ")
                nc.sync.dma_start(
                    out=candQ[:].rearrange("(p u) two -> p (u two)", p=128),
                    in_=cG[:])
                # exact merge: 3 rounds of top-8 on the strided value view
                sv = cG[:].rearrange("p (u two) -> p u two", two=2)[:, :, 0]
                gvals = pool.tile([128, 24], F32, tag="gvals")
                gpos = pool.tile([128, 24], U32, tag="gpos")
                for r in range(3):
                    m8 = gvals[:, r * 8:(r + 1) * 8]
                    nc.vector.max(out=m8, in_=sv)
                    nc.vector.max_index(
                        out=gpos[:, r * 8:(r + 1) * 8], in_max=m8, in_values=sv)
                    if r < 2:
                        nc.vector.match_replace(
                            out=sv, in_to_replace=m8, in_values=sv, imm_value=NEG)
                # softmax over the top-20 values
                negm = pool.tile([128, 1], F32, tag="negm")
                nc.vector.tensor_scalar(
                    negm[:], gvals[:, 0:1], -1.0, None, op0=mybir.AluOpType.mult)
                wexp = pool.tile([128, TOPK], F32, tag="wexp")
                ssum = pool.tile([128, 1], F32, tag="ssum")
                nc.scalar.activation(
                    out=wexp[:], in_=gvals[:, :TOPK],
                    func=mybir.ActivationFunctionType.Exp,
                    bias=negm[:], scale=1.0, accum_out=ssum[:])
                rs = pool.tile([128, 1], F32, tag="rs")
                nc.vector.reciprocal(rs[:], ssum[:])
                wgt = pool.tile([128, TOPK], F32, tag="wgt")
                nc.vector.tensor_scalar(
                    wgt[:], wexp[:], rs[:], None, op0=mybir.AluOpType.mult)
                # winner pair offsets: row p of candQ-pairs = p*256 + pos
                posf = pool.tile([128, 24], F32, tag="posf")
                nc.vector.tensor_copy(posf[:], gpos[:])
                nc.vector.tensor_scalar(
                    posf[:], posf[:], pr256[:], None, op0=mybir.AluOpType.add)
                pou = pool.tile([128, 24], U32, tag="pou")
                nc.vector.tensor_copy(pou[:], posf[:])
                acc = pool.tile([128, 2 * CV], F32, tag="acc")
                nc.vector.memset(acc[:], 0.0)
                for k in range(TOPK):
                    pk = deep.tile([128, 2], F32, tag="pk")
                    nc.gpsimd.indirect_dma_start(
                        out=pk[:], out_offset=None, in_=candQ[:],
                        in_offset=bass.IndirectOffsetOnAxis(
                            ap=pou[:, k:k + 1], axis=0))
                    iku = deep.tile([128, 1], U32, tag="iku")
                    nc.scalar.copy(out=iku[:], in_=pk[:, 1:2])
                    gk = deep.tile([128, 2 * CV], F32, tag="gk")
                    nc.gpsimd.indirect_dma_start(
                        out=gk[:], out_offset=None, in_=vT[:],
                        in_offset=bass.IndirectOffsetOnAxis(ap=iku[:], axis=0))
                    nc.vector.scalar_tensor_tensor(
                        out=acc[:], in0=gk[:], scalar=wgt[:, k:k + 1], in1=acc[:],
                        op0=mybir.AluOpType.mult, op1=mybir.AluOpType.add)
                nc.sync.dma_start(
                    out=out[tt * 128:(tt + 1) * 128, :], in_=acc[:])


            # ---------------- Phase 1: local affinity + per-group top-8 ----
            # tile order: chunk-major (j, d) with t = d*NQT3 + j so each
            # chunk's AllToAll can fire as soon as its 8 tiles are done
            _order = [d * NQT3 + j for j in range(NQT3) for d in range(NC)]
            for ti, t in enumerate(_order[:NQT if "1" in phases else 0]):
                affs = affp.tile([128, NE_LOC], F32, tag="affs")
                cvals = pool.tile([128, NCAND], F32, tag="cvals", bufs=4)
                cidx = pool.tile([128, NCAND], U32, tag="cidx", bufs=4)
                for h in range(2):
                    ph = psum.tile([128, NE_LOC // 2], F32, tag="ph")
                    for c in range(4):
                        nc.tensor.matmul(
                            out=ph[:, c * 512:(c + 1) * 512],
                            lhsT=qt[:, t * 128:(t + 1) * 128],
                            rhs=mkt[:, h * 2048 + c * 512: h * 2048 + (c + 1) * 512],
                            start=True, stop=True)
                    nc.scalar.copy(out=affs[:, h * 2048:(h + 1) * 2048], in_=ph[:])
                    for g in range(NGRP):
                        if not (GB[g] < (h + 1) * 2048 and GB[g + 1] > h * 2048
                                and GB[g + 1] <= (h + 1) * 2048):
                            continue
                        sl = affs[:, GB[g]:GB[g + 1]]
                        nc.vector.max(out=cvals[:, g * 8:(g + 1) * 8], in_=sl)
                        nc.vector.max_index(
                            out=cidx[:, g * 8:(g + 1) * 8],
                            in_max=cvals[:, g * 8:(g + 1) * 8], in_values=sl)
                crow = pool.tile([128, 2 * NCAND], F32, tag="crow", bufs=4)
                cr3 = crow[:].rearrange("p (u two) -> p u two", two=2)
                nc.scalar.copy(out=cr3[:, :, 0], in_=cvals[:])
                nc.vector.scalar_tensor_tensor(
                    out=cr3[:, :, 1], in0=cidx[:], scalar=1.0, in1=gb[:],
                    op0=mybir.AluOpType.mult, op1=mybir.AluOpType.add)
                j, d = t % NQT3, t // NQT3
                row = (j * NC + d) * 128
                nc.sync.dma_start(
                    out=candL[row:row + 128, :], in_=crow[:])
                if "2" in phases and ti % NC == NC - 1:
                    nc.gpsimd.collective_compute(
                        "AllToAll", mybir.AluOpType.bypass,
                        replica_groups=[list(range(NC))],
                        ins=[candL[j * NC * 128:(j + 1) * NC * 128, :].opt()],
                        outs=[candX[j * NC * 128:(j + 1) * NC * 128, :].opt()])
                    if "3" in phases:
                        phase3(j)


            if "3" not in phases:
                dummy = pool.tile([128, 2 * CV], F32, tag="dummy")
                nc.vector.memset(dummy[:], 0.0)
                for tt in range(NQT3):
                    nc.sync.dma_start(out=out[tt * 128:(tt + 1) * 128, :], in_=dummy[:])
            if "1" not in phases:
                # phase3-only: candL must still exist for collective; fill zero
                z = pool.tile([128, 2 * NCAND], F32, tag="z")
                nc.vector.memset(z[:], 0.0)
                for t in range(NQT):
                    nc.sync.dma_start(out=candL[t * 128:(t + 1) * 128, :], in_=z[:])
    nc.finalize()
    _prog_cache[phases] = nc
    return nc


def kernel(qk, mem_k, mem_v1, mem_v2, top_k):
    assert int(top_k) == TOPK
    qk = np.asarray(qk, dtype=np.float32)
    mem_k = np.asarray(mem_k, dtype=np.float32)
    mem_v1 = np.asarray(mem_v1, dtype=np.float32)
    mem_v2 = np.asarray(mem_v2, dtype=np.float32)

    q2 = qk.reshape(CK, Q)
    qTa = np.concatenate([q2 * 0.25, np.ones((1, Q), np.float32)], axis=0)
    a = np.sum(mem_k[0] * mem_k[0], axis=0, dtype=np.float32)  # [NE]
    vT = np.concatenate([mem_v1[0].T, mem_v2[0].T], axis=1).copy()  # [NE, 512]
    gbase = np.repeat(np.array(GB[:NGRP], dtype=np.float32), 8)
    prow256 = (np.arange(128, dtype=np.float32) * NSLOT).reshape(128, 1)

    in_maps = []
    for c in range(NC):
        sl = slice(c * NE_LOC, (c + 1) * NE_LOC)
        mkA = np.concatenate(
            [mem_k[0][:, sl], (-0.125 * a[sl])[None, :]], axis=0)
        in_maps.append({
            "qTa": qTa, "mkA": np.ascontiguousarray(mkA), "vT": vT,
            "gnc": np.broadcast_to(
                gbase + c * NE_LOC, (128, NCAND)).astype(np.float32).copy(),
            "prow256": prow256,
        })

    nc = _build_program()
    res = None
    for attempt in range(3):
        try:
            res = run_bass_kernel_spmd(nc, in_maps, core_ids=list(range(NC)))
            break
        except Exception:
            # transient device-unrecoverable states clear on the next attempt
            if attempt == 2:
                raise
            time.sleep(2.0)
    full = np.concatenate([res.results[c]["out"] for c in range(NC)], axis=0)
    return np.ascontiguousarray(full.T).reshape(1, 2 * CV, H, W)



# revision 6
# speedup vs baseline: 1.0969x; 1.0969x over previous
import sys, time
sys.path.insert(0, "/opt/trn_rl_repo")
import numpy as np
from concourse import bass, bacc, mybir, tile
from concourse.bass_utils import run_bass_kernel_spmd

# Problem constants (nn_Memory_88656714925588)
B, CK, CV = 1, 64, 256
H, W, T = 64, 64, 8
NE = H * W * T            # 32768
Q = H * W                 # 4096
NC = 8                    # cores
NE_LOC = NE // NC         # 4096 memory elements per core
Q_LOC = Q // NC           # 512 queries per core in phase 3
TOPK = 20
NGRP = 2                  # groups per query-tile (one per PSUM half)
GW = 2048                 # group width
NCAND = NGRP * 8          # 16 candidates per (query, core)
NPAIR = NC * NCAND        # 128 candidate pairs per query after exchange
NQT = Q // 128            # 32 query tiles in phase 1
NQT3 = Q_LOC // 128       # 4 query tiles per core in phase 3
F32 = mybir.dt.float32
import os
F32R = mybir.dt.float32 if os.environ.get("K_NO_F32R") else mybir.dt.float32r
F16 = mybir.dt.float16
U32 = mybir.dt.uint32
NEG = -1e30
COPY = mybir.ActivationFunctionType.Copy

_prog_cache = {}


def _build_program(phases="123"):
    if phases in _prog_cache:
        return _prog_cache[phases]
    nc = bacc.Bacc()
    qTa = nc.dram_tensor("qTa", [CK + 1, Q], F32R, kind="ExternalInput")
    mkA = nc.dram_tensor("mkA", [CK + 1, NE_LOC], F32R, kind="ExternalInput")
    vTh = nc.dram_tensor("vTh", [NE, 2 * CV], F16, kind="ExternalInput")
    gbt = nc.dram_tensor("gbt", [128, NGRP], F32, kind="ExternalInput")
    prow = nc.dram_tensor("prow", [128, 1], F32, kind="ExternalInput")
    out = nc.dram_tensor("out", [Q_LOC, 2 * CV], F16, kind="ExternalOutput")

    with tile.TileContext(nc) as tc:
        with tc.tile_pool(name="sbuf", bufs=2) as pool, \
             tc.tile_pool(name="deep", bufs=3) as deep, \
             tc.tile_pool(name="cst", bufs=1) as cst, \
             tc.tile_pool(name="psum", bufs=2, space="PSUM") as psum, \
             tc.tile_pool(name="dram", bufs=2, space="DRAM") as dram:

            qt = cst.tile([CK + 1, Q], F32R)
            mkt = cst.tile([CK + 1, NE_LOC], F32R)
            # chunked loads: first matmul needs only mkt[:, :512] and
            # qt[:, :128], so let compute start before the full MB lands
            for ci in range(8):
                nc.sync.dma_start(
                    out=mkt[:, ci * 512:(ci + 1) * 512],
                    in_=mkA[:, ci * 512:(ci + 1) * 512])
            for ci in range(4):
                nc.sync.dma_start(
                    out=qt[:, ci * 1024:(ci + 1) * 1024],
                    in_=qTa[:, ci * 1024:(ci + 1) * 1024])
            gb = cst.tile([128, NGRP], F32)
            nc.sync.dma_start(out=gb[:], in_=gbt[:])
            pr = cst.tile([128, 1], F32)
            nc.sync.dma_start(out=pr[:], in_=prow[:])

            candL = dram.tile([Q, 2 * NCAND], F32)
            candX = dram.tile([Q, 2 * NCAND], F32)

            # ---------------- Phase 3: merge + readout (q-sharded) --------
            def phase3(tt):
                cG = pool.tile([128, NC * 2 * NCAND], F32, tag="cG")
                nc.sync.dma_start(
                    out=cG[:],
                    in_=candX[tt * NC * 128:(tt + 1) * NC * 128, :]
                    .rearrange("(g p) c -> p g c", p=128))
                candQ = dram.tile([128 * NPAIR, 2], F32, tag="candQ")
                nc.sync.dma_start(
                    out=candQ[:].rearrange("(p u) two -> p (u two)", p=128),
                    in_=cG[:])
                # exact merge: 3 rounds of top-8 on the strided value view
                sv = cG[:].rearrange("p (u two) -> p u two", two=2)[:, :, 0]
                gvals = pool.tile([128, 24], F32, tag="gvals")
                gpos = pool.tile([128, 24], U32, tag="gpos")
                for r in range(3):
                    m8 = gvals[:, r * 8:(r + 1) * 8]
                    nc.vector.max(out=m8, in_=sv)
                    nc.vector.max_index(
                        out=gpos[:, r * 8:(r + 1) * 8], in_max=m8, in_values=sv)
                    if r < 2:
                        nc.vector.match_replace(
                            out=sv, in_to_replace=m8, in_values=sv, imm_value=NEG)
                # softmax over the top-20 values (Act engine)
                negm = pool.tile([128, 1], F32, tag="negm")
                nc.scalar.activation(
                    out=negm[:], in_=gvals[:, 0:1], func=COPY,
                    bias=0.0, scale=-1.0)
                wexp = pool.tile([128, TOPK], F32, tag="wexp")
                ssum = pool.tile([128, 1], F32, tag="ssum")
                nc.scalar.activation(
                    out=wexp[:], in_=gvals[:, :TOPK],
                    func=mybir.ActivationFunctionType.Exp,
                    bias=negm[:], scale=1.0, accum_out=ssum[:])
                rs = pool.tile([128, 1], F32, tag="rs")
                nc.vector.reciprocal(rs[:], ssum[:])
                wgt = pool.tile([128, TOPK], F32, tag="wgt")
                nc.scalar.activation(
                    out=wgt[:], in_=wexp[:], func=COPY, bias=0.0, scale=rs[:])
                # winner pair offsets: row p of candQ-pairs = p*NPAIR + pos
                posf = pool.tile([128, TOPK], F32, tag="posf")
                nc.scalar.activation(
                    out=posf[:], in_=gpos[:, :TOPK],
                    func=mybir.ActivationFunctionType.Relu,
                    bias=pr[:], scale=1.0)
                pou = pool.tile([128, TOPK], U32, tag="pou")
                nc.scalar.copy(out=pou[:], in_=posf[:])
                # batched pair gather: (val, idx) for all 20 winners at once
                pk = pool.tile([128, TOPK, 2], F32, tag="pk")
                if os.environ.get("K_GATHER_LOOP"):
                    for k in range(TOPK):
                        nc.gpsimd.indirect_dma_start(
                            out=pk[:, k, :], out_offset=None, in_=candQ[:],
                            in_offset=bass.IndirectOffsetOnAxis(
                                ap=pou[:, k:k + 1], axis=0))
                else:
                    nc.gpsimd.indirect_dma_start(
                        out=pk[:].rearrange("p k two -> p (k two)"),
                        out_offset=None, in_=candQ[:],
                        in_offset=bass.IndirectOffsetOnAxis(ap=pou[:], axis=0))
                iku = pool.tile([128, TOPK], U32, tag="iku")
                nc.scalar.copy(out=iku[:], in_=pk[:, :, 1])
                # batched value-row gather (fp16): 20 rows of 512 per query
                gk = pool.tile([128, TOPK, 2 * CV], F16, tag="gk")
                if os.environ.get("K_GATHER_LOOP"):
                    for k in range(TOPK):
                        nc.gpsimd.indirect_dma_start(
                            out=gk[:, k, :], out_offset=None, in_=vTh[:],
                            in_offset=bass.IndirectOffsetOnAxis(
                                ap=iku[:, k:k + 1], axis=0))
                else:
                    nc.gpsimd.indirect_dma_start(
                        out=gk[:].rearrange("p k c -> p (k c)"),
                        out_offset=None, in_=vTh[:],
                        in_offset=bass.IndirectOffsetOnAxis(ap=iku[:], axis=0))
                # weighted sum: Act scales each row, DVE accumulates (fp16)
                acc = pool.tile([128, 2 * CV], F16, tag="acc")
                gs0 = deep.tile([128, 2 * CV], F16, tag="gs0")
                nc.scalar.activation(
                    out=acc[:], in_=gk[:, 0, :], func=COPY,
                    bias=0.0, scale=wgt[:, 0:1])
                for k in range(1, TOPK):
                    gs = deep.tile([128, 2 * CV], F16, tag="gs")
                    nc.scalar.activation(
                        out=gs[:], in_=gk[:, k, :], func=COPY,
                        bias=0.0, scale=wgt[:, k:k + 1])
                    nc.vector.tensor_tensor(
                        out=acc[:], in0=acc[:], in1=gs[:],
                        op=mybir.AluOpType.add)
                nc.sync.dma_start(
                    out=out[tt * 128:(tt + 1) * 128, :], in_=acc[:])

            # ---------------- Phase 1: local affinity + per-group top-8 ----
            # tile order: chunk-major (j, d) with t = d*NQT3 + j so each
            # chunk's AllToAll can fire as soon as its 8 tiles are done
            _order = [d * NQT3 + j for j in range(NQT3) for d in range(NC)]
            for ti, t in enumerate(_order[:NQT if "1" in phases else 0]):
                crow = pool.tile([128, 2 * NCAND], F32, tag="crow", bufs=4)
                for g in range(NGRP):
                    ph = psum.tile([128, GW], F32, tag="ph")
                    for c in range(4):
                        nc.tensor.matmul(
                            out=ph[:, c * 512:(c + 1) * 512],
                            lhsT=qt[:, t * 128:(t + 1) * 128],
                            rhs=mkt[:, g * GW + c * 512: g * GW + (c + 1) * 512],
                            start=True, stop=True)
                    # top-8 of this group directly from PSUM; values land
                    # interleaved in crow, indices cast+based on Act
                    cv8 = crow[:].rearrange(
                        "p (u two) -> p u two", two=2)[:, g * 8:(g + 1) * 8, 0]
                    nc.vector.max(out=cv8, in_=ph[:])
                    cidx = pool.tile([128, 8], U32, tag="cidx", bufs=4)
                    nc.vector.max_index(
                        out=cidx[:], in_max=cv8, in_values=ph[:])
                    nc.scalar.activation(
                        out=crow[:].rearrange(
                            "p (u two) -> p u two", two=2)[:, g * 8:(g + 1) * 8, 1],
                        in_=cidx[:],
                        func=mybir.ActivationFunctionType.Relu,
                        bias=gb[:, g:g + 1], scale=1.0)
                j, d = t % NQT3, t // NQT3
                row = (j * NC + d) * 128
                nc.sync.dma_start(
                    out=candL[row:row + 128, :], in_=crow[:])
                if "2" in phases and ti % NC == NC - 1:
                    nc.gpsimd.collective_compute(
                        "AllToAll", mybir.AluOpType.bypass,
                        replica_groups=[list(range(NC))],
                        ins=[candL[j * NC * 128:(j + 1) * NC * 128, :].opt()],
                        outs=[candX[j * NC * 128:(j + 1) * NC * 128, :].opt()])
                    if "3" in phases:
                        phase3(j)

            if "3" not in phases:
                dummy = pool.tile([128, 2 * CV], F16, tag="dummy")
                nc.vector.memset(dummy[:], 0.0)
                for tt in range(NQT3):
                    nc.sync.dma_start(out=out[tt * 128:(tt + 1) * 128, :], in_=dummy[:])
            if "1" not in phases:
                z = pool.tile([128, 2 * NCAND], F32, tag="z")
                nc.vector.memset(z[:], 0.0)
                for t in range(NQT):
                    nc.sync.dma_start(out=candL[t * 128:(t + 1) * 128, :], in_=z[:])
    nc.finalize()
    _prog_cache[phases] = nc
    return nc


def _host_inputs(qk, mem_k, mem_v1, mem_v2):
    q2 = qk.reshape(CK, Q)
    qTa = np.concatenate([q2 * 0.25, np.ones((1, Q), np.float32)], axis=0)
    a = np.sum(mem_k[0] * mem_k[0], axis=0, dtype=np.float32)  # [NE]
    vTh = np.concatenate(
        [mem_v1[0].T, mem_v2[0].T], axis=1).astype(np.float16)  # [NE, 512]
    prow = (np.arange(128, dtype=np.float32) * NPAIR).reshape(128, 1)

    in_maps = []
    for c in range(NC):
        sl = slice(c * NE_LOC, (c + 1) * NE_LOC)
        mkA = np.concatenate(
            [mem_k[0][:, sl], (-0.125 * a[sl])[None, :]], axis=0)
        gbt = np.broadcast_to(
            np.array([c * NE_LOC + g * GW for g in range(NGRP)],
                     dtype=np.float32), (128, NGRP)).copy()
        in_maps.append({
            "qTa": qTa, "mkA": np.ascontiguousarray(mkA), "vTh": vTh,
            "gbt": gbt, "prow": prow,
        })
    return in_maps


def kernel(qk, mem_k, mem_v1, mem_v2, top_k):
    assert int(top_k) == TOPK
    qk = np.asarray(qk, dtype=np.float32)
    mem_k = np.asarray(mem_k, dtype=np.float32)
    mem_v1 = np.asarray(mem_v1, dtype=np.float32)
    mem_v2 = np.asarray(mem_v2, dtype=np.float32)

    in_maps = _host_inputs(qk, mem_k, mem_v1, mem_v2)
    nc = _build_program()
    res = None
    for attempt in range(3):
        try:
            res = run_bass_kernel_spmd(nc, in_maps, core_ids=list(range(NC)))
            break
        except Exception:
            # transient device-unrecoverable states clear on the next attempt
            if attempt == 2:
                raise
            time.sleep(2.0)
    full = np.concatenate(
        [res.results[c]["out"].astype(np.float32) for c in range(NC)], axis=0)
    return np.ascontiguousarray(full.T).reshape(1, 2 * CV, H, W)


# revision 12
# speedup vs baseline: 1.1105x; 1.0123x over previous
import sys, time
sys.path.insert(0, "/opt/trn_rl_repo")
import numpy as np
from concourse import bass, bacc, mybir, tile
from concourse.bass_utils import run_bass_kernel_spmd

# Problem constants (nn_Memory_88656714925588)
B, CK, CV = 1, 64, 256
H, W, T = 64, 64, 8
NE = H * W * T            # 32768
Q = H * W                 # 4096
NC = 8                    # cores
NE_LOC = NE // NC         # 4096 memory elements per core
Q_LOC = Q // NC           # 512 queries per core in phase 3
TOPK = 20
NGRP = 2                  # groups per query-tile (one per PSUM half)
GW = 2048                 # group width
NCAND = NGRP * 8          # 16 candidates per (query, core)
NPAIR = NC * NCAND        # 128 candidate pairs per query after exchange
NQT = Q // 128            # 32 query tiles in phase 1
NQT3 = Q_LOC // 128       # 4 query tiles per core in phase 3
F32 = mybir.dt.float32
import os
# float32r matmuls produced garbage on real hardware via the axon/PJRT
# path; keep plain fp32 unless explicitly re-enabled for experiments.
F32R = mybir.dt.float32r if os.environ.get("K_F32R") else mybir.dt.float32
F16 = mybir.dt.float16
U32 = mybir.dt.uint32
NEG = -1e30
COPY = mybir.ActivationFunctionType.Copy

_prog_cache = {}


def _build_program(phases="123"):
    if phases in _prog_cache:
        return _prog_cache[phases]
    nc = bacc.Bacc()
    qTa = nc.dram_tensor("qTa", [CK + 1, Q], F32R, kind="ExternalInput")
    mkA = nc.dram_tensor("mkA", [CK + 1, NE_LOC], F32R, kind="ExternalInput")
    vTh = nc.dram_tensor("vTh", [NE, 2 * CV], F16, kind="ExternalInput")
    gbt = nc.dram_tensor("gbt", [128, NGRP], F32, kind="ExternalInput")
    prow = nc.dram_tensor("prow", [128, 1], F32, kind="ExternalInput")
    out = nc.dram_tensor("out", [Q_LOC, 2 * CV], F16, kind="ExternalOutput")

    with tile.TileContext(nc) as tc:
        with tc.tile_pool(name="sbuf", bufs=2) as pool, \
             tc.tile_pool(name="deep", bufs=3) as deep, \
             tc.tile_pool(name="cst", bufs=1) as cst, \
             tc.tile_pool(name="psum", bufs=2, space="PSUM") as psum, \
             tc.tile_pool(name="dram", bufs=2, space="DRAM") as dram:

            qt = cst.tile([CK + 1, Q], F32R)
            mkt = cst.tile([CK + 1, NE_LOC], F32R)
            # chunked loads: first matmul needs only mkt[:, :512] and
            # qt[:, :128], so let compute start before the full MB lands
            for ci in range(8):
                nc.sync.dma_start(
                    out=mkt[:, ci * 512:(ci + 1) * 512],
                    in_=mkA[:, ci * 512:(ci + 1) * 512])
            for ci in range(4):
                nc.sync.dma_start(
                    out=qt[:, ci * 1024:(ci + 1) * 1024],
                    in_=qTa[:, ci * 1024:(ci + 1) * 1024])
            gb = cst.tile([128, NGRP], F32)
            nc.sync.dma_start(out=gb[:], in_=gbt[:])
            pr = cst.tile([128, 1], F32)
            nc.sync.dma_start(out=pr[:], in_=prow[:])

            candL = dram.tile([Q, 2 * NCAND], F32)
            candX = dram.tile([Q, 2 * NCAND], F32)

            # ---------------- Phase 3: merge + readout (q-sharded) --------
            def phase3(tt):
                cG = pool.tile([128, NC * 2 * NCAND], F32, tag="cG")
                nc.sync.dma_start(
                    out=cG[:],
                    in_=candX[tt * NC * 128:(tt + 1) * NC * 128, :]
                    .rearrange("(g p) c -> p g c", p=128))
                candQ = dram.tile([128 * NPAIR, 2], F32, tag="candQ")
                nc.sync.dma_start(
                    out=candQ[:].rearrange("(p u) two -> p (u two)", p=128),
                    in_=cG[:])
                # exact merge: 3 rounds of top-8 on the strided value view
                sv = cG[:].rearrange("p (u two) -> p u two", two=2)[:, :, 0]
                gvals = pool.tile([128, 24], F32, tag="gvals")
                gpos = pool.tile([128, 24], U32, tag="gpos")
                for r in range(3):
                    m8 = gvals[:, r * 8:(r + 1) * 8]
                    nc.vector.max(out=m8, in_=sv)
                    nc.vector.max_index(
                        out=gpos[:, r * 8:(r + 1) * 8], in_max=m8, in_values=sv)
                    if r < 2:
                        nc.vector.match_replace(
                            out=sv, in_to_replace=m8, in_values=sv, imm_value=NEG)
                # softmax over the top-20 values (Act engine)
                negm = pool.tile([128, 1], F32, tag="negm")
                nc.scalar.activation(
                    out=negm[:], in_=gvals[:, 0:1], func=COPY,
                    bias=0.0, scale=-1.0)
                wexp = pool.tile([128, TOPK], F32, tag="wexp")
                ssum = pool.tile([128, 1], F32, tag="ssum")
                nc.scalar.activation(
                    out=wexp[:], in_=gvals[:, :TOPK],
                    func=mybir.ActivationFunctionType.Exp,
                    bias=negm[:], scale=1.0, accum_out=ssum[:])
                rs = pool.tile([128, 1], F32, tag="rs")
                nc.vector.reciprocal(rs[:], ssum[:])
                wgt = pool.tile([128, TOPK], F32, tag="wgt")
                nc.scalar.activation(
                    out=wgt[:], in_=wexp[:], func=COPY, bias=0.0, scale=rs[:])
                # winner pair offsets: row p of candQ-pairs = p*NPAIR + pos
                posf = pool.tile([128, TOPK], F32, tag="posf")
                nc.scalar.activation(
                    out=posf[:], in_=gpos[:, :TOPK],
                    func=mybir.ActivationFunctionType.Relu,
                    bias=pr[:], scale=1.0)
                pou = pool.tile([128, TOPK], U32, tag="pou")
                nc.scalar.copy(out=pou[:], in_=posf[:])
                # batched pair gather: (val, idx) for all 20 winners at once
                # multi-offset (batched) indirect gathers scramble data on
                # real hardware, so gather per winner: first the (val, idx)
                # pair, then the fp16 value row.
                pk = pool.tile([128, TOPK, 2], F32, tag="pk")
                if os.environ.get("K_BATCH_PK"):
                    nc.gpsimd.indirect_dma_start(
                        out=pk[:].rearrange("p k two -> p (k two)"),
                        out_offset=None, in_=candQ[:],
                        in_offset=bass.IndirectOffsetOnAxis(ap=pou[:], axis=0))
                else:
                    for k in range(TOPK):
                        nc.gpsimd.indirect_dma_start(
                            out=pk[:, k, :], out_offset=None, in_=candQ[:],
                            in_offset=bass.IndirectOffsetOnAxis(
                                ap=pou[:, k:k + 1], axis=0))
                iku = pool.tile([128, TOPK], U32, tag="iku")
                nc.scalar.copy(out=iku[:], in_=pk[:, :, 1])
                gk = pool.tile([128, TOPK, 2 * CV], F16, tag="gk")
                if os.environ.get("K_BATCH_GK"):
                    nc.gpsimd.indirect_dma_start(
                        out=gk[:].rearrange("p k c -> p (k c)"),
                        out_offset=None, in_=vTh[:],
                        in_offset=bass.IndirectOffsetOnAxis(ap=iku[:], axis=0))
                else:
                    for k in range(TOPK):
                        nc.gpsimd.indirect_dma_start(
                            out=gk[:, k, :], out_offset=None, in_=vTh[:],
                            in_offset=bass.IndirectOffsetOnAxis(
                                ap=iku[:, k:k + 1], axis=0))
                # weighted sum: Act scales each row, DVE accumulates (fp16)
                acc = pool.tile([128, 2 * CV], F16, tag="acc")
                nc.scalar.activation(
                    out=acc[:], in_=gk[:, 0, :], func=COPY,
                    bias=0.0, scale=wgt[:, 0:1])
                for k in range(1, TOPK):
                    gs = deep.tile([128, 2 * CV], F16, tag="gs")
                    nc.scalar.activation(
                        out=gs[:], in_=gk[:, k, :], func=COPY,
                        bias=0.0, scale=wgt[:, k:k + 1])
                    nc.vector.tensor_tensor(
                        out=acc[:], in0=acc[:], in1=gs[:],
                        op=mybir.AluOpType.add)
                nc.sync.dma_start(
                    out=out[tt * 128:(tt + 1) * 128, :], in_=acc[:])

            # ---------------- Phase 1: local affinity + per-group top-8 ----
            # tile order: chunk-major (j, d) with t = d*NQT3 + j so each
            # chunk's AllToAll can fire as soon as its 8 tiles are done
            _order = [d * NQT3 + j for j in range(NQT3) for d in range(NC)]
            for ti, t in enumerate(_order[:NQT if "1" in phases else 0]):
                crow = pool.tile([128, 2 * NCAND], F32, tag="crow", bufs=4)
                for g in range(NGRP):
                    ph = psum.tile([128, GW], F32, tag="ph")
                    for c in range(4):
                        nc.tensor.matmul(
                            out=ph[:, c * 512:(c + 1) * 512],
                            lhsT=qt[:, t * 128:(t + 1) * 128],
                            rhs=mkt[:, g * GW + c * 512: g * GW + (c + 1) * 512],
                            start=True, stop=True)
                    # Act drains PSUM to SBUF (frees the bank early and the
                    # DVE scans pay SBUF not PSUM access latency)
                    affs = pool.tile([128, GW], F32, tag="affs", bufs=3)
                    nc.scalar.copy(out=affs[:], in_=ph[:])
                    # top-8 of this group; values land interleaved in crow,
                    # indices cast+based on Act
                    cv8 = crow[:].rearrange(
                        "p (u two) -> p u two", two=2)[:, g * 8:(g + 1) * 8, 0]
                    nc.vector.max(out=cv8, in_=affs[:])
                    cidx = pool.tile([128, 8], U32, tag="cidx", bufs=4)
                    nc.vector.max_index(
                        out=cidx[:], in_max=cv8, in_values=affs[:])
                    nc.scalar.activation(
                        out=crow[:].rearrange(
                            "p (u two) -> p u two", two=2)[:, g * 8:(g + 1) * 8, 1],
                        in_=cidx[:],
                        func=mybir.ActivationFunctionType.Relu,
                        bias=gb[:, g:g + 1], scale=1.0)
                j, d = t % NQT3, t // NQT3
                row = (j * NC + d) * 128
                nc.sync.dma_start(
                    out=candL[row:row + 128, :], in_=crow[:])
                if "2" in phases and ti % NC == NC - 1:
                    nc.gpsimd.collective_compute(
                        "AllToAll", mybir.AluOpType.bypass,
                        replica_groups=[list(range(NC))],
                        ins=[candL[j * NC * 128:(j + 1) * NC * 128, :].opt()],
                        outs=[candX[j * NC * 128:(j + 1) * NC * 128, :].opt()])
                    if "3" in phases:
                        phase3(j)

            if "3" not in phases:
                dummy = pool.tile([128, 2 * CV], F16, tag="dummy")
                nc.vector.memset(dummy[:], 0.0)
                for tt in range(NQT3):
                    nc.sync.dma_start(out=out[tt * 128:(tt + 1) * 128, :], in_=dummy[:])
            if "1" not in phases:
                z = pool.tile([128, 2 * NCAND], F32, tag="z")
                nc.vector.memset(z[:], 0.0)
                for t in range(NQT):
                    nc.sync.dma_start(out=candL[t * 128:(t + 1) * 128, :], in_=z[:])
    nc.finalize()
    _prog_cache[phases] = nc
    return nc


def _host_inputs(qk, mem_k, mem_v1, mem_v2):
    q2 = qk.reshape(CK, Q)
    qTa = np.concatenate([q2 * 0.25, np.ones((1, Q), np.float32)], axis=0)
    a = np.sum(mem_k[0] * mem_k[0], axis=0, dtype=np.float32)  # [NE]
    vTh = np.concatenate(
        [mem_v1[0].T, mem_v2[0].T], axis=1).astype(np.float16)  # [NE, 512]
    prow = (np.arange(128, dtype=np.float32) * NPAIR).reshape(128, 1)

    in_maps = []
    for c in range(NC):
        sl = slice(c * NE_LOC, (c + 1) * NE_LOC)
        mkA = np.concatenate(
            [mem_k[0][:, sl], (-0.125 * a[sl])[None, :]], axis=0)
        gbt = np.broadcast_to(
            np.array([c * NE_LOC + g * GW for g in range(NGRP)],
                     dtype=np.float32), (128, NGRP)).copy()
        in_maps.append({
            "qTa": qTa, "mkA": np.ascontiguousarray(mkA), "vTh": vTh,
            "gbt": gbt, "prow": prow,
        })
    return in_maps


def kernel(qk, mem_k, mem_v1, mem_v2, top_k):
    assert int(top_k) == TOPK
    qk = np.asarray(qk, dtype=np.float32)
    mem_k = np.asarray(mem_k, dtype=np.float32)
    mem_v1 = np.asarray(mem_v1, dtype=np.float32)
    mem_v2 = np.asarray(mem_v2, dtype=np.float32)

    in_maps = _host_inputs(qk, mem_k, mem_v1, mem_v2)
    nc = _build_program()
    res = None
    for attempt in range(3):
        try:
            res = run_bass_kernel_spmd(nc, in_maps, core_ids=list(range(NC)))
            break
        except Exception:
            # transient device-unrecoverable states clear on the next attempt
            if attempt == 2:
                raise
            time.sleep(2.0)
    full = np.concatenate(
        [res.results[c]["out"].astype(np.float32) for c in range(NC)], axis=0)
    return np.ascontiguousarray(full.T).reshape(1, 2 * CV, H, W)


# revision 14
# speedup vs baseline: 1.1341x; 1.0213x over previous
import sys, time
sys.path.insert(0, "/opt/trn_rl_repo")
import numpy as np
from concourse import bass, bacc, mybir, tile
from concourse.bass_utils import run_bass_kernel_spmd

# Problem constants (nn_Memory_88656714925588)
B, CK, CV = 1, 64, 256
H, W, T = 64, 64, 8
NE = H * W * T            # 32768
Q = H * W                 # 4096
NC = 8                    # cores
NE_LOC = NE // NC         # 4096 memory elements per core
Q_LOC = Q // NC           # 512 queries per core in phase 3
TOPK = 20
NGRP = 2                  # groups per query-tile (one per PSUM half)
GW = 2048                 # group width
NCAND = NGRP * 8          # 16 candidates per (query, core)
NPAIR = NC * NCAND        # 128 candidate pairs per query after exchange
NQT = Q // 128            # 32 query tiles in phase 1
NQT3 = Q_LOC // 128       # 4 query tiles per core in phase 3
F32 = mybir.dt.float32
import os
# float32r matmuls produced garbage on real hardware via the axon/PJRT
# path; keep plain fp32 unless explicitly re-enabled for experiments.
F32R = mybir.dt.float32r if os.environ.get("K_F32R") else mybir.dt.float32
F16 = mybir.dt.float16
U32 = mybir.dt.uint32
NEG = -1e30
COPY = mybir.ActivationFunctionType.Copy

_prog_cache = {}


def _build_program(phases="123"):
    if phases in _prog_cache:
        return _prog_cache[phases]
    nc = bacc.Bacc()
    qTa = nc.dram_tensor("qTa", [CK + 1, Q], F32R, kind="ExternalInput")
    mkA = nc.dram_tensor("mkA", [CK + 1, NE_LOC], F32R, kind="ExternalInput")
    vTh = nc.dram_tensor("vTh", [NE, 2 * CV], F16, kind="ExternalInput")
    gbt = nc.dram_tensor("gbt", [128, NGRP], F32, kind="ExternalInput")
    prow = nc.dram_tensor("prow", [128, 1], F32, kind="ExternalInput")
    out = nc.dram_tensor("out", [Q_LOC, 2 * CV], F16, kind="ExternalOutput")

    with tile.TileContext(nc) as tc:
        with tc.tile_pool(name="sbuf", bufs=2) as pool, \
             tc.tile_pool(name="deep", bufs=3) as deep, \
             tc.tile_pool(name="cst", bufs=1) as cst, \
             tc.tile_pool(name="psum", bufs=2, space="PSUM") as psum, \
             tc.tile_pool(name="dram", bufs=2, space="DRAM") as dram:

            qt = cst.tile([CK + 1, Q], F32R)
            mkt = cst.tile([CK + 1, NE_LOC], F32R)
            # chunked loads: first matmul needs only mkt[:, :512] and
            # qt[:, :128], so let compute start before the full MB lands
            for ci in range(8):
                nc.sync.dma_start(
                    out=mkt[:, ci * 512:(ci + 1) * 512],
                    in_=mkA[:, ci * 512:(ci + 1) * 512])
            for ci in range(4):
                nc.sync.dma_start(
                    out=qt[:, ci * 1024:(ci + 1) * 1024],
                    in_=qTa[:, ci * 1024:(ci + 1) * 1024])
            gb = cst.tile([128, NGRP], F32)
            nc.sync.dma_start(out=gb[:], in_=gbt[:])
            pr = cst.tile([128, 1], F32)
            nc.sync.dma_start(out=pr[:], in_=prow[:])

            candL = dram.tile([Q, 2 * NCAND], F32)
            candX = dram.tile([Q, 2 * NCAND], F32)

            # ---------------- Phase 3: merge + readout (q-sharded) --------
            def phase3(tt, tail=False):
                cG = pool.tile([128, NC * 2 * NCAND], F32, tag="cG")
                nc.sync.dma_start(
                    out=cG[:],
                    in_=candX[tt * NC * 128:(tt + 1) * NC * 128, :]
                    .rearrange("(g p) c -> p g c", p=128))
                candQ = dram.tile([128 * NPAIR, 2], F32, tag="candQ")
                nc.sync.dma_start(
                    out=candQ[:].rearrange("(p u) two -> p (u two)", p=128),
                    in_=cG[:])
                # exact merge: 3 rounds of top-8 on the strided value view
                sv = cG[:].rearrange("p (u two) -> p u two", two=2)[:, :, 0]
                gvals = pool.tile([128, 24], F32, tag="gvals")
                gpos = pool.tile([128, 24], U32, tag="gpos")
                for r in range(3):
                    m8 = gvals[:, r * 8:(r + 1) * 8]
                    nc.vector.max(out=m8, in_=sv)
                    nc.vector.max_index(
                        out=gpos[:, r * 8:(r + 1) * 8], in_max=m8, in_values=sv)
                    if r < 2:
                        nc.vector.match_replace(
                            out=sv, in_to_replace=m8, in_values=sv, imm_value=NEG)
                # softmax over the top-20 values (Act engine)
                negm = pool.tile([128, 1], F32, tag="negm")
                nc.scalar.activation(
                    out=negm[:], in_=gvals[:, 0:1], func=COPY,
                    bias=0.0, scale=-1.0)
                wexp = pool.tile([128, TOPK], F32, tag="wexp")
                ssum = pool.tile([128, 1], F32, tag="ssum")
                nc.scalar.activation(
                    out=wexp[:], in_=gvals[:, :TOPK],
                    func=mybir.ActivationFunctionType.Exp,
                    bias=negm[:], scale=1.0, accum_out=ssum[:])
                rs = pool.tile([128, 1], F32, tag="rs")
                nc.vector.reciprocal(rs[:], ssum[:])
                wgt = pool.tile([128, TOPK], F32, tag="wgt")
                nc.scalar.activation(
                    out=wgt[:], in_=wexp[:], func=COPY, bias=0.0, scale=rs[:])
                iku = pool.tile([128, TOPK], U32, tag="iku")
                if tail:
                    # phase-1 work is exhausted after the last chunk, so the
                    # DVE is idle: extract winner indices with per-partition
                    # window reductions instead of a serial Pool gather chain
                    civ = cG[:].rearrange("p (u two) -> p u two", two=2)[:, :, 1]
                    posr = pool.tile([128, TOPK], F32, tag="posr")
                    nc.scalar.copy(out=posr[:], in_=gpos[:, :TOPK])
                    pose = pool.tile([128, TOPK], F32, tag="pose")
                    nc.scalar.activation(
                        out=pose[:], in_=gpos[:, :TOPK],
                        func=mybir.ActivationFunctionType.Relu,
                        bias=1.0, scale=1.0)
                    idxf = pool.tile([128, TOPK], F32, tag="idxf")
                    mrs = pool.tile([128, NPAIR], F32, tag="mrs")
                    for k in range(TOPK):
                        nc.vector.tensor_mask_reduce(
                            out=mrs[:], in_=civ,
                            mask_start=posr[:, k:k + 1],
                            mask_end=pose[:, k:k + 1],
                            scale=1.0, accum_in=0.0,
                            op=mybir.AluOpType.max,
                            accum_out=idxf[:, k:k + 1])
                    nc.scalar.copy(out=iku[:], in_=idxf[:])
                else:
                    # winner pair offsets: row p of candQ-pairs = p*NPAIR+pos
                    posf = pool.tile([128, TOPK], F32, tag="posf")
                    nc.scalar.activation(
                        out=posf[:], in_=gpos[:, :TOPK],
                        func=mybir.ActivationFunctionType.Relu,
                        bias=pr[:], scale=1.0)
                    pou = pool.tile([128, TOPK], U32, tag="pou")
                    nc.scalar.copy(out=pou[:], in_=posf[:])
                    # multi-offset (batched) indirect gathers scramble data
                    # on real hardware, so gather (val, idx) per winner
                    pk = pool.tile([128, TOPK, 2], F32, tag="pk")
                    for k in range(TOPK):
                        nc.gpsimd.indirect_dma_start(
                            out=pk[:, k, :], out_offset=None, in_=candQ[:],
                            in_offset=bass.IndirectOffsetOnAxis(
                                ap=pou[:, k:k + 1], axis=0))
                    nc.scalar.copy(out=iku[:], in_=pk[:, :, 1])
                gk = pool.tile([128, TOPK, 2 * CV], F16, tag="gk")
                for k in range(TOPK):
                    nc.gpsimd.indirect_dma_start(
                        out=gk[:, k, :], out_offset=None, in_=vTh[:],
                        in_offset=bass.IndirectOffsetOnAxis(
                            ap=iku[:, k:k + 1], axis=0))
                # weighted sum: Act scales each row, DVE accumulates (fp16)
                acc = pool.tile([128, 2 * CV], F16, tag="acc")
                nc.scalar.activation(
                    out=acc[:], in_=gk[:, 0, :], func=COPY,
                    bias=0.0, scale=wgt[:, 0:1])
                for k in range(1, TOPK):
                    gs = deep.tile([128, 2 * CV], F16, tag="gs")
                    nc.scalar.activation(
                        out=gs[:], in_=gk[:, k, :], func=COPY,
                        bias=0.0, scale=wgt[:, k:k + 1])
                    nc.vector.tensor_tensor(
                        out=acc[:], in0=acc[:], in1=gs[:],
                        op=mybir.AluOpType.add)
                nc.sync.dma_start(
                    out=out[tt * 128:(tt + 1) * 128, :], in_=acc[:])

            # ---------------- Phase 1: local affinity + per-group top-8 ----
            # tile order: chunk-major (j, d) with t = d*NQT3 + j so each
            # chunk's AllToAll can fire as soon as its 8 tiles are done
            _order = [d * NQT3 + j for j in range(NQT3) for d in range(NC)]
            for ti, t in enumerate(_order[:NQT if "1" in phases else 0]):
                crow = pool.tile([128, 2 * NCAND], F32, tag="crow", bufs=4)
                for g in range(NGRP):
                    ph = psum.tile([128, GW], F32, tag="ph")
                    for c in range(4):
                        nc.tensor.matmul(
                            out=ph[:, c * 512:(c + 1) * 512],
                            lhsT=qt[:, t * 128:(t + 1) * 128],
                            rhs=mkt[:, g * GW + c * 512: g * GW + (c + 1) * 512],
                            start=True, stop=True)
                    # Act drains PSUM to SBUF (frees the bank early and the
                    # DVE scans pay SBUF not PSUM access latency)
                    affs = pool.tile([128, GW], F32, tag="affs", bufs=3)
                    nc.scalar.copy(out=affs[:], in_=ph[:])
                    # top-8 of this group; values land interleaved in crow,
                    # indices cast+based on Act
                    cv8 = crow[:].rearrange(
                        "p (u two) -> p u two", two=2)[:, g * 8:(g + 1) * 8, 0]
                    nc.vector.max(out=cv8, in_=affs[:])
                    cidx = pool.tile([128, 8], U32, tag="cidx", bufs=4)
                    nc.vector.max_index(
                        out=cidx[:], in_max=cv8, in_values=affs[:])
                    nc.scalar.activation(
                        out=crow[:].rearrange(
                            "p (u two) -> p u two", two=2)[:, g * 8:(g + 1) * 8, 1],
                        in_=cidx[:],
                        func=mybir.ActivationFunctionType.Relu,
                        bias=gb[:, g:g + 1], scale=1.0)
                j, d = t % NQT3, t // NQT3
                row = (j * NC + d) * 128
                nc.sync.dma_start(
                    out=candL[row:row + 128, :], in_=crow[:])
                if "2" in phases and ti % NC == NC - 1:
                    nc.gpsimd.collective_compute(
                        "AllToAll", mybir.AluOpType.bypass,
                        replica_groups=[list(range(NC))],
                        ins=[candL[j * NC * 128:(j + 1) * NC * 128, :].opt()],
                        outs=[candX[j * NC * 128:(j + 1) * NC * 128, :].opt()])
                    if "3" in phases:
                        phase3(j, tail=(ti == NQT - 1))

            if "3" not in phases:
                dummy = pool.tile([128, 2 * CV], F16, tag="dummy")
                nc.vector.memset(dummy[:], 0.0)
                for tt in range(NQT3):
                    nc.sync.dma_start(out=out[tt * 128:(tt + 1) * 128, :], in_=dummy[:])
            if "1" not in phases:
                z = pool.tile([128, 2 * NCAND], F32, tag="z")
                nc.vector.memset(z[:], 0.0)
                for t in range(NQT):
                    nc.sync.dma_start(out=candL[t * 128:(t + 1) * 128, :], in_=z[:])
    nc.finalize()
    _prog_cache[phases] = nc
    return nc


def _host_inputs(qk, mem_k, mem_v1, mem_v2):
    q2 = qk.reshape(CK, Q)
    qTa = np.concatenate([q2 * 0.25, np.ones((1, Q), np.float32)], axis=0)
    a = np.sum(mem_k[0] * mem_k[0], axis=0, dtype=np.float32)  # [NE]
    vTh = np.concatenate(
        [mem_v1[0].T, mem_v2[0].T], axis=1).astype(np.float16)  # [NE, 512]
    prow = (np.arange(128, dtype=np.float32) * NPAIR).reshape(128, 1)

    in_maps = []
    for c in range(NC):
        sl = slice(c * NE_LOC, (c + 1) * NE_LOC)
        mkA = np.concatenate(
            [mem_k[0][:, sl], (-0.125 * a[sl])[None, :]], axis=0)
        gbt = np.broadcast_to(
            np.array([c * NE_LOC + g * GW for g in range(NGRP)],
                     dtype=np.float32), (128, NGRP)).copy()
        in_maps.append({
            "qTa": qTa, "mkA": np.ascontiguousarray(mkA), "vTh": vTh,
            "gbt": gbt, "prow": prow,
        })
    return in_maps


def kernel(qk, mem_k, mem_v1, mem_v2, top_k):
    assert int(top_k) == TOPK
    qk = np.asarray(qk, dtype=np.float32)
    mem_k = np.asarray(mem_k, dtype=np.float32)
    mem_v1 = np.asarray(mem_v1, dtype=np.float32)
    mem_v2 = np.asarray(mem_v2, dtype=np.float32)

    in_maps = _host_inputs(qk, mem_k, mem_v1, mem_v2)
    nc = _build_program()
    res = None
    for attempt in range(3):
        try:
            res = run_bass_kernel_spmd(nc, in_maps, core_ids=list(range(NC)))
            break
        except Exception:
            # transient device-unrecoverable states clear on the next attempt
            if attempt == 2:
                raise
            time.sleep(2.0)
    full = np.concatenate(
        [res.results[c]["out"].astype(np.float32) for c in range(NC)], axis=0)
    return np.ascontiguousarray(full.T).reshape(1, 2 * CV, H, W)
